# revision 43
# baseline (speedup 1.0000x reference)
"""AttnBlock Trainium2 Bass kernel.

Data-parallel over batch across 8 NeuronCores (4 batch elements each, full
weights on every core). Everything on-chip is feature-major ([feat, token]),
so the pipeline needs no transposes anywhere.

The kernel is paced by the Scalar engine: softmax exp is 16.8M elements per
core and ACT runs 1 elem/lane/cycle @1.2GHz => ~147us floor (128 ACT ops).
The whole design exists to keep that stream gapless:

  - scores: sp [128, 2, 512] psum, both heads x one i-half, head pair
    row-tiled (tile_position (0,0)/(64,0)) so it streams concurrently;
    two sp tiles ping-pong. NOTHING else ever touches the sp slots, so
    the scores->exp stream has no foreign WAR waits.
  - ctx: ONE [65, N] psum accumulator (row 64 = softmax Z via the ones
    column of V). Each attention unit's ctx matmuls are REPLAYED from the
    SBUF P tiles during the NEXT unit, spread 4 MMs per jc-slot into the
    PE's slack under the ACT pace. Head h0 replays in jc0-3, h1 in jc4-7.
  - projections (QK/V/out) run in a DEDICATED fq psum slot, emitted as
    one filler per jc-slot; their DVE consumers chain only to each other.
  - normalize: copies + reciprocal (pure DVE) right at each head's replay
    end; the GPSIMD broadcast then fills; the multiplies are deferred two
    jc-slots so they never wait on GPSIMD from inside the DVE FIFO.

PSUM (8 banks, exactly full): sp0, sp1 (4), cH (2), fq (2).

Matmul operands are bf16 (converted host-side; fp32 PSUM accumulation).
"""

import numpy as np
import ml_dtypes

N_HEADS = 4
D_K = 64
SCALE = D_K ** (-0.5)
B, C, H, W = 32, 256, 32, 32
N = H * W           # 1024 tokens
NCORES = 8
BPC = B // NCORES   # 4 batch elements per core

_CACHE = {}


def _build():
    import concourse.bacc as bacc
    import concourse.mybir as mybir
    from concourse.tile import TileContext

    dt = mybir.dt
    f32 = dt.float32
    bf16 = dt.bfloat16
    EXP = mybir.ActivationFunctionType.Exp
    ADD = mybir.AluOpType.add
    MULT = mybir.AluOpType.mult

    nc = bacc.Bacc()
    x = nc.dram_tensor("x", [BPC, C, N], f32, kind="ExternalInput")
    xbf = nc.dram_tensor("xbf", [BPC, C, N], bf16, kind="ExternalInput")
    wqk = nc.dram_tensor("wqk", [C, 512], bf16, kind="ExternalInput")
    bqk = nc.dram_tensor("bqk", [128, 4], f32, kind="ExternalInput")
    wv = nc.dram_tensor("wv", [C, 260], bf16, kind="ExternalInput")
    wvb2 = nc.dram_tensor("wvb2", [128, 520], f32, kind="ExternalInput")
    wo = nc.dram_tensor("wo", [C, C], bf16, kind="ExternalInput")
    ob = nc.dram_tensor("ob", [128, 2], f32, kind="ExternalInput")
    out = nc.dram_tensor("out", [BPC, C, N], f32, kind="ExternalOutput")

    with TileContext(nc) as tc:
        with (
            tc.tile_pool(name="consts", bufs=1) as consts,
            tc.tile_pool(name="xp", bufs=4) as xp,
            tc.tile_pool(name="qkp", bufs=5) as qkp,
            tc.tile_pool(name="vp", bufs=3) as vp,
            tc.tile_pool(name="pp", bufs=26) as pp,
            tc.tile_pool(name="miscp", bufs=4) as miscp,
            tc.tile_pool(name="outp", bufs=2) as outp,
            tc.tile_pool(name="psum", bufs=1, space="PSUM") as psum,
        ):
            wqk_sb = [consts.tile([128, 512], bf16, name=f"wqk{cc}") for cc in range(2)]
            wv_sb = [consts.tile([128, 260], bf16, name=f"wv{cc}") for cc in range(2)]
            wo_sb = [consts.tile([128, 256], bf16, name=f"wo{cc}") for cc in range(2)]
            bqk_sb = consts.tile([128, 4], f32, name="bqk_sb")
            wvb2_sb = consts.tile([128, 520], f32, name="wvb2_sb")
            ob_sb = consts.tile([128, 2], f32, name="ob_sb")
            nc.sync.dma_start(out=bqk_sb[:], in_=bqk[:])
            for cc in range(2):
                nc.sync.dma_start(out=wqk_sb[cc][:], in_=wqk[cc * 128:(cc + 1) * 128, :])
                nc.sync.dma_start(out=wv_sb[cc][:], in_=wv[cc * 128:(cc + 1) * 128, :])
                nc.sync.dma_start(out=wo_sb[cc][:], in_=wo[cc * 128:(cc + 1) * 128, :])
            nc.sync.dma_start(out=wvb2_sb[:], in_=wvb2[:])
            nc.sync.dma_start(out=ob_sb[:], in_=ob[:])
            warmup = consts.tile([1, 4], f32, name="warmup")
            nc.scalar.activation(warmup[:], bqk_sb[0:1, 0:4], EXP)

            xcs, xcrs, qks, vss, cns, pts = {}, {}, {}, {}, {}, {}

            def emit_x_load(b):
                xc = [xp.tile([128, N], f32, name=f"xc{cc}", tag=f"xc{cc}", bufs=3)
                      for cc in range(2)]
                xcr = [xp.tile([128, N], bf16, name=f"xcr{cc}", tag=f"xcr{cc}", bufs=3)
                       for cc in range(2)]
                for cc in range(2):
                    nc.sync.dma_start(out=xcr[cc][:], in_=xbf[b, cc * 128:(cc + 1) * 128, :])
                for cc in range(2):
                    nc.sync.dma_start(out=xc[cc][:], in_=x[b, cc * 128:(cc + 1) * 128, :])
                xcs[b] = xc
                xcrs[b] = xcr

            def emit_qk_full(b, p, qk, tag="fq"):
                if b not in qks:
                    qks[b] = [[None, None], [None, None]]
                xcr = xcrs[b]
                qkps = psum.tile([128, N], f32, name="qkps", tag=tag)
                col0 = p * 256 + qk * 128
                for fc in range(2):
                    fs = slice(fc * 512, (fc + 1) * 512)
                    for cc in range(2):
                        nc.tensor.matmul(
                            qkps[:, fs],
                            wqk_sb[cc][:, col0:col0 + 128],
                            xcr[cc][:, fs],
                            start=(cc == 0), stop=(cc == 1),
                        )
                qks[b][p][qk] = qkp.tile([128, N], bf16, name=f"qk{p}{qk}")
                nc.vector.tensor_scalar(
                    qks[b][p][qk][:], qkps[:],
                    bqk_sb[:, 2 * p + qk:2 * p + qk + 1],
                    None, ADD,
                )

            def emit_v_pair(b, pr, tag="fq"):
                if b not in vss:
                    vss[b] = vp.tile([128, 8, 260], bf16, name="v_sb", tag="v")
                xcr = xcrs[b]
                vps = psum.tile([128, 2, 512], f32, name="vps", tag=tag)
                for k in range(2):
                    jt = 2 * pr + k
                    js = slice(jt * 128, (jt + 1) * 128)
                    for cc in range(2):
                        nc.tensor.matmul(
                            vps[:, k, 0:260],
                            xcr[cc][:, js], wv_sb[cc][:],
                            start=(cc == 0), stop=(cc == 1),
                        )
                nc.vector.scalar_tensor_tensor(
                    vss[b][:, 2 * pr:2 * pr + 2, :], vps[:, :, 0:260],
                    1.0, wvb2_sb[:], MULT, ADD,
                )

            def emit_outproj_full(b, co):
                osb = outp.tile([128, N], f32, name="osb")
                ctxn = cns[b]
                ops = psum.tile([128, N], f32, name="ops", tag="fq")
                for fc in range(2):
                    fs = slice(fc * 512, (fc + 1) * 512)
                    for kc in range(2):
                        nc.tensor.matmul(
                            ops[:, fs],
                            wo_sb[kc][:, co * 128:(co + 1) * 128],
                            ctxn[kc][:, fs],
                            start=(kc == 0), stop=(kc == 1),
                        )
                nc.vector.scalar_tensor_tensor(
                    osb[:], ops[:], ob_sb[:, co:co + 1], xcs[b][co][:],
                    ADD, ADD,
                )
                nc.sync.dma_start(
                    out=out[b, co * 128:(co + 1) * 128, :], in_=osb[:]
                )

            # ---- ctx replay machinery -------------------------------------
            # replaying unit u=(b,p): head h's 16 ctx MMs accumulate into a
            # fresh cH tile; afterwards phase-A normalize (copies+recip,
            # pure DVE) releases cH and seeds the GPSIMD broadcast; the
            # multiply closure is returned for deferred emission.
            state = {}

            def replay_mms(u, hl, part, tag="c"):
                b, p = u
                h = 2 * p + hl
                v_sb = vss[b]
                if part == 0:
                    state[tag] = psum.tile([65, N], f32, name="cH", tag=tag)
                cH = state[tag]
                for k in range(4):
                    idx = part * 4 + k
                    jc, ic = idx // 2, idx % 2
                    isl = slice(ic * 512, (ic + 1) * 512)
                    nc.tensor.matmul(
                        cH[:, isl],
                        v_sb[:, jc, h * 65:(h + 1) * 65],
                        pts[u][(jc, ic)][:, hl, :],
                        start=(jc == 0), stop=(jc == 7),
                    )

            def norm_a(u, hl, tag="c"):
                b, p = u
                if hl == 0:
                    cns[u[0]] = cns.get(u[0]) or [None, None]
                if cns[b][p] is None:
                    cns[b][p] = miscp.tile([128, N], bf16, name=f"cn{p}",
                                           tag="cn", bufs=3)
                cn = cns[b][p]
                cH = state[tag]
                cu = miscp.tile([65, N], f32, name="cu", tag="cu", bufs=4)
                nc.vector.tensor_copy(cu[:], cH[:])
                z_sb = miscp.tile([1, N], f32, name="z_sb", tag="z", bufs=1)
                nc.vector.tensor_copy(z_sb[:], cu[64:65, :])
                rz = miscp.tile([1, N], f32, name="rz", tag="rz", bufs=2)
                nc.vector.reciprocal_approx_fast(rz[:], z_sb[:])
                rzb = miscp.tile([64, N], f32, name="rzb", tag="rzb", bufs=3)
                nc.gpsimd.partition_broadcast(rzb[:], rz[0:1, :])

                def mult(cn=cn, hl=hl, cu=cu, rzb=rzb):
                    nc.vector.tensor_tensor(
                        cn[hl * 64:(hl + 1) * 64, :],
                        cu[0:64, :],
                        rzb[:],
                        MULT,
                    )
                return mult

            def emit_pack(u, fillers):
                b, p = u
                qst, kst = qks[b][p][0], qks[b][p][1]
                pts[u] = {}
                for jc in range(8):
                    js = slice(jc * 128, (jc + 1) * 128)
                    for ic in range(2):
                        isl = slice(ic * 512, (ic + 1) * 512)
                        sp = psum.tile([128, 2, 512], f32, name="sp",
                                       tag=f"sp{ic}")
                        for hl in range(2):
                            hs = slice(hl * 64, (hl + 1) * 64)
                            nc.tensor.matmul(
                                sp[:, hl, :],
                                kst[hs, js],
                                qst[hs, isl],
                                start=True, stop=True,
                                tile_position=(hl * 64, 0),
                            )
                        pt = pp.tile([128, 2, 512], bf16, name="pt", tag="pt")
                        nc.scalar.activation(pt[:], sp[:], EXP, scale=SCALE)
                        pts[u][(jc, ic)] = pt
                    for f in fillers[jc]:
                        f()

            def chainf(*fns):
                return [f for f in fns if f is not None]

            units = [(b, p) for b in range(BPC) for p in range(2)]

            # prologue
            emit_x_load(0)
            emit_qk_full(0, 0, 0, tag="sp0")
            emit_qk_full(0, 0, 1, tag="sp1")
            for pr in range(4):
                emit_v_pair(0, pr, tag=("fq", "sp0", "sp1", "fq")[pr])
            emit_x_load(1)

            mult_q = []   # deferred normalize multiplies
            for ui, u in enumerate(units):
                b, p = u
                prev_u = units[ui - 1] if ui >= 1 else None
                nxt_b = b + 1 if b + 1 < BPC else None

                def rp(hl, part, pu=prev_u):
                    if pu is None:
                        return None
                    return lambda: replay_mms(pu, hl, part)

                def na(hl, pu=prev_u):
                    if pu is None:
                        return None
                    return lambda: mult_q.append(norm_a(pu, hl))

                def dm():
                    # drain one deferred multiply
                    return (lambda: mult_q.pop(0)()) if mult_q else None

                if p == 0:
                    projf = [
                        None,
                        None,
                        (lambda _b=b: emit_qk_full(_b, 1, 0)),
                        (lambda _b=b: emit_qk_full(_b, 1, 1)),
                        (lambda _n=nxt_b: emit_v_pair(_n, 0)) if nxt_b is not None else None,
                        None,
                        (lambda _n=nxt_b: emit_v_pair(_n, 1)) if nxt_b is not None else None,
                        None,
                    ]
                else:
                    prv_b = b - 1 if b >= 1 else None
                    last_u = (ui == len(units) - 1)
                    projf = [
                        None,
                        None,
                        (lambda _p=prv_b: emit_outproj_full(_p, 0)) if prv_b is not None else None,
                        (lambda _p=prv_b: emit_outproj_full(_p, 1)) if prv_b is not None else None,
                        (lambda _n=nxt_b: emit_v_pair(_n, 2)) if nxt_b is not None else None,
                        (lambda _n=nxt_b: emit_qk_full(_n, 0, 0)) if nxt_b is not None else None,
                        (lambda _n=nxt_b: emit_v_pair(_n, 3)) if nxt_b is not None else None,
                        (lambda _n=nxt_b: emit_qk_full(_n, 0, 1)) if nxt_b is not None else None,
                    ]
                    if last_u:
                        # overlap the final unit's h0 replay with its own
                        # attention stream, in the now-idle fq slot
                        projf[4] = (lambda _u=u: replay_mms(_u, 0, 0, tag="fq"))
                        projf[5] = (lambda _u=u: replay_mms(_u, 0, 1, tag="fq"))
                        projf[6] = (lambda _u=u: replay_mms(_u, 0, 2, tag="fq"))
                        projf[7] = (lambda _u=u: [replay_mms(_u, 0, 3, tag="fq"),
                                                  mult_q.append(norm_a(_u, 0, tag="fq"))])

                fillers = [
                    chainf(rp(0, 0), dm(), projf[0]),
                    chainf(rp(0, 1), dm(), projf[1]),
                    chainf(rp(0, 2), projf[2]),
                    chainf(rp(0, 3), na(0), projf[3]),
                    chainf(rp(1, 0), projf[4]),
                    chainf(rp(1, 1), projf[5]),
                    chainf(rp(1, 2), projf[6]),
                    chainf(rp(1, 3), na(1), projf[7]),
                ]
                emit_pack(u, fillers)
                if p == 1 and b + 2 < BPC:
                    emit_x_load(b + 2)

            # tail: replay the final unit, finish norms, last out projection
            last = units[-1]
            for part in range(4):
                replay_mms(last, 1, part)
            mult_q.append(norm_a(last, 1))
            for f in mult_q:
                f()
            for co in range(2):
                emit_outproj_full(BPC - 1, co)

    nc.compile()
    return nc


def _prep_weights(proj_w, proj_b, out_w, out_b):
    qk_cols = []
    for p in range(2):
        for qk in range(2):
            for hl in range(2):
                h = 2 * p + hl
                base = h * 192 + qk * 64
                qk_cols.extend(range(base, base + 64))
    wqk = np.ascontiguousarray(proj_w[qk_cols, :].T).astype(ml_dtypes.bfloat16)
    bqk = np.ascontiguousarray(proj_b[qk_cols].reshape(4, 128).T)

    wv = np.zeros((C, 260), dtype=np.float32)
    wvb1 = np.zeros((1, 260), dtype=np.float32)
    for h in range(N_HEADS):
        rows = range(h * 192 + 128, h * 192 + 192)
        wv[:, h * 65:h * 65 + 64] = proj_w[rows, :].T
        wvb1[0, h * 65:h * 65 + 64] = proj_b[rows]
        wvb1[0, h * 65 + 64] = 1.0
    wv = wv.astype(ml_dtypes.bfloat16)
    wvb2 = np.ascontiguousarray(
        np.broadcast_to(np.concatenate([wvb1, wvb1], axis=1), (128, 520))
    )

    wo = np.ascontiguousarray(out_w.T).astype(ml_dtypes.bfloat16)
    ob = np.ascontiguousarray(out_b.reshape(2, 128).T)
    return dict(wqk=wqk, bqk=bqk, wv=wv, wvb2=wvb2, wo=wo, ob=ob)


def kernel(x, proj_w, proj_b, out_w, out_b, _trace=False):
    from concourse.bass_utils import run_bass_kernel_spmd

    x = np.asarray(x, dtype=np.float32)
    proj_w = np.asarray(proj_w, dtype=np.float32)
    proj_b = np.asarray(proj_b, dtype=np.float32)
    out_w = np.asarray(out_w, dtype=np.float32)
    out_b = np.asarray(out_b, dtype=np.float32)

    if "nc" not in _CACHE:
        _CACHE["nc"] = _build()
    nc = _CACHE["nc"]

    w = _prep_weights(proj_w, proj_b, out_w, out_b)
    xs = np.ascontiguousarray(x.reshape(B, C, N))
    xsbf = xs.astype(ml_dtypes.bfloat16)
    in_maps = [
        dict(w, x=np.ascontiguousarray(xs[i * BPC:(i + 1) * BPC]),
             xbf=np.ascontiguousarray(xsbf[i * BPC:(i + 1) * BPC]))
        for i in range(NCORES)
    ]
    res = run_bass_kernel_spmd(nc, in_maps, core_ids=list(range(NCORES)), trace=_trace)
    out = np.concatenate([r["out"] for r in res.results], axis=0)
    out = out.reshape(B, C, H, W)
    if _trace:
        _CACHE["last_result"] = res
    return out


# revision 44
# speedup vs baseline: 1.0005x; 1.0005x over previous
"""AttnBlock Trainium2 Bass kernel.

Data-parallel over batch across 8 NeuronCores (4 batch elements each, full
weights on every core). Everything on-chip is feature-major ([feat, token]),
so the pipeline needs no transposes anywhere.

The kernel is paced by the Scalar engine: softmax exp is 16.8M elements per
core and ACT runs 1 elem/lane/cycle @1.2GHz => ~147us floor (128 ACT ops).
The whole design exists to keep that stream gapless:

  - scores: sp [128, 2, 512] psum, both heads x one i-half, head pair
    row-tiled (tile_position (0,0)/(64,0)) so it streams concurrently;
    two sp tiles ping-pong. NOTHING else ever touches the sp slots, so
    the scores->exp stream has no foreign WAR waits.
  - ctx: ONE [65, N] psum accumulator (row 64 = softmax Z via the ones
    column of V). Each attention unit's ctx matmuls are REPLAYED from the
    SBUF P tiles during the NEXT unit, spread 4 MMs per jc-slot into the
    PE's slack under the ACT pace. Head h0 replays in jc0-3, h1 in jc4-7.
  - projections (QK/V/out) run in a DEDICATED fq psum slot, emitted as
    one filler per jc-slot; their DVE consumers chain only to each other.
  - normalize: copies + reciprocal (pure DVE) right at each head's replay
    end; the GPSIMD broadcast then fills; the multiplies are deferred two
    jc-slots so they never wait on GPSIMD from inside the DVE FIFO.

PSUM (8 banks, exactly full): sp0, sp1 (4), cH (2), fq (2).

Matmul operands are bf16 (converted host-side; fp32 PSUM accumulation).
"""

import numpy as np
import ml_dtypes

N_HEADS = 4
D_K = 64
SCALE = D_K ** (-0.5)
B, C, H, W = 32, 256, 32, 32
N = H * W           # 1024 tokens
NCORES = 8
BPC = B // NCORES   # 4 batch elements per core

_CACHE = {}


def _build():
    import concourse.bacc as bacc
    import concourse.mybir as mybir
    from concourse.tile import TileContext

    dt = mybir.dt
    f32 = dt.float32
    bf16 = dt.bfloat16
    EXP = mybir.ActivationFunctionType.Exp
    ADD = mybir.AluOpType.add
    MULT = mybir.AluOpType.mult

    nc = bacc.Bacc()
    x = nc.dram_tensor("x", [BPC, C, N], f32, kind="ExternalInput")
    xbf = nc.dram_tensor("xbf", [BPC, C, N], bf16, kind="ExternalInput")
    wqk = nc.dram_tensor("wqk", [C, 512], bf16, kind="ExternalInput")
    bqk = nc.dram_tensor("bqk", [128, 4], f32, kind="ExternalInput")
    wv = nc.dram_tensor("wv", [C, 260], bf16, kind="ExternalInput")
    wvb2 = nc.dram_tensor("wvb2", [128, 520], f32, kind="ExternalInput")
    wo = nc.dram_tensor("wo", [C, C], bf16, kind="ExternalInput")
    ob = nc.dram_tensor("ob", [128, 2], f32, kind="ExternalInput")
    out = nc.dram_tensor("out", [BPC, C, N], f32, kind="ExternalOutput")

    with TileContext(nc) as tc:
        with (
            tc.tile_pool(name="consts", bufs=1) as consts,
            tc.tile_pool(name="xp", bufs=4) as xp,
            tc.tile_pool(name="qkp", bufs=5) as qkp,
            tc.tile_pool(name="vp", bufs=3) as vp,
            tc.tile_pool(name="pp", bufs=26) as pp,
            tc.tile_pool(name="miscp", bufs=4) as miscp,
            tc.tile_pool(name="outp", bufs=2) as outp,
            tc.tile_pool(name="psum", bufs=1, space="PSUM") as psum,
        ):
            wqk_sb = [consts.tile([128, 512], bf16, name=f"wqk{cc}") for cc in range(2)]
            wv_sb = [consts.tile([128, 260], bf16, name=f"wv{cc}") for cc in range(2)]
            wo_sb = [consts.tile([128, 256], bf16, name=f"wo{cc}") for cc in range(2)]
            bqk_sb = consts.tile([128, 4], f32, name="bqk_sb")
            wvb2_sb = consts.tile([128, 520], f32, name="wvb2_sb")
            ob_sb = consts.tile([128, 2], f32, name="ob_sb")
            nc.sync.dma_start(out=bqk_sb[:], in_=bqk[:])
            for cc in range(2):
                nc.sync.dma_start(out=wqk_sb[cc][:], in_=wqk[cc * 128:(cc + 1) * 128, :])
                nc.sync.dma_start(out=wv_sb[cc][:], in_=wv[cc * 128:(cc + 1) * 128, :])
                nc.sync.dma_start(out=wo_sb[cc][:], in_=wo[cc * 128:(cc + 1) * 128, :])
            nc.sync.dma_start(out=wvb2_sb[:], in_=wvb2[:])
            nc.sync.dma_start(out=ob_sb[:], in_=ob[:])
            warmup = consts.tile([1, 4], f32, name="warmup")
            nc.scalar.activation(warmup[:], bqk_sb[0:1, 0:4], EXP)

            xcs, xcrs, qks, vss, cns, pts = {}, {}, {}, {}, {}, {}

            def emit_x_load(b):
                xc = [xp.tile([128, N], f32, name=f"xc{cc}", tag=f"xc{cc}", bufs=3)
                      for cc in range(2)]
                xcr = [xp.tile([128, N], bf16, name=f"xcr{cc}", tag=f"xcr{cc}", bufs=3)
                       for cc in range(2)]
                for cc in range(2):
                    nc.sync.dma_start(out=xcr[cc][:], in_=xbf[b, cc * 128:(cc + 1) * 128, :])
                for cc in range(2):
                    nc.sync.dma_start(out=xc[cc][:], in_=x[b, cc * 128:(cc + 1) * 128, :])
                xcs[b] = xc
                xcrs[b] = xcr

            def emit_qk_full(b, p, qk, tag="fq"):
                if b not in qks:
                    qks[b] = [[None, None], [None, None]]
                xcr = xcrs[b]
                qkps = psum.tile([128, N], f32, name="qkps", tag=tag)
                col0 = p * 256 + qk * 128
                for fc in range(2):
                    fs = slice(fc * 512, (fc + 1) * 512)
                    for cc in range(2):
                        nc.tensor.matmul(
                            qkps[:, fs],
                            wqk_sb[cc][:, col0:col0 + 128],
                            xcr[cc][:, fs],
                            start=(cc == 0), stop=(cc == 1),
                        )
                qks[b][p][qk] = qkp.tile([128, N], bf16, name=f"qk{p}{qk}")
                nc.vector.tensor_scalar(
                    qks[b][p][qk][:], qkps[:],
                    bqk_sb[:, 2 * p + qk:2 * p + qk + 1],
                    None, ADD,
                )

            def emit_v_pair(b, pr, tag="fq"):
                if b not in vss:
                    vss[b] = vp.tile([128, 8, 260], bf16, name="v_sb", tag="v")
                xcr = xcrs[b]
                vps = psum.tile([128, 2, 512], f32, name="vps", tag=tag)
                for k in range(2):
                    jt = 2 * pr + k
                    js = slice(jt * 128, (jt + 1) * 128)
                    for cc in range(2):
                        nc.tensor.matmul(
                            vps[:, k, 0:260],
                            xcr[cc][:, js], wv_sb[cc][:],
                            start=(cc == 0), stop=(cc == 1),
                        )
                nc.vector.scalar_tensor_tensor(
                    vss[b][:, 2 * pr:2 * pr + 2, :], vps[:, :, 0:260],
                    1.0, wvb2_sb[:], MULT, ADD,
                )

            def emit_outproj_full(b, co):
                osb = outp.tile([128, N], f32, name="osb")
                ctxn = cns[b]
                ops = psum.tile([128, N], f32, name="ops", tag="fq")
                for fc in range(2):
                    fs = slice(fc * 512, (fc + 1) * 512)
                    for kc in range(2):
                        nc.tensor.matmul(
                            ops[:, fs],
                            wo_sb[kc][:, co * 128:(co + 1) * 128],
                            ctxn[kc][:, fs],
                            start=(kc == 0), stop=(kc == 1),
                        )
                nc.vector.scalar_tensor_tensor(
                    osb[:], ops[:], ob_sb[:, co:co + 1], xcs[b][co][:],
                    ADD, ADD,
                )
                nc.sync.dma_start(
                    out=out[b, co * 128:(co + 1) * 128, :], in_=osb[:]
                )

            # ---- ctx replay machinery -------------------------------------
            # replaying unit u=(b,p): head h's 16 ctx MMs accumulate into a
            # fresh cH tile; afterwards phase-A normalize (copies+recip,
            # pure DVE) releases cH and seeds the GPSIMD broadcast; the
            # multiply closure is returned for deferred emission.
            state = {}

            def replay_mms(u, hl, part, tag="c"):
                b, p = u
                h = 2 * p + hl
                v_sb = vss[b]
                if part == 0:
                    state[tag] = psum.tile([65, N], f32, name="cH", tag=tag)
                cH = state[tag]
                for k in range(4):
                    idx = part * 4 + k
                    jc, ic = idx // 2, idx % 2
                    isl = slice(ic * 512, (ic + 1) * 512)
                    nc.tensor.matmul(
                        cH[:, isl],
                        v_sb[:, jc, h * 65:(h + 1) * 65],
                        pts[u][(jc, ic)][:, hl, :],
                        start=(jc == 0), stop=(jc == 7),
                    )

            def norm_a(u, hl, tag="c"):
                b, p = u
                if hl == 0:
                    cns[u[0]] = cns.get(u[0]) or [None, None]
                if cns[b][p] is None:
                    cns[b][p] = miscp.tile([128, N], bf16, name=f"cn{p}",
                                           tag="cn", bufs=3)
                cn = cns[b][p]
                cH = state[tag]
                cu = miscp.tile([65, N], f32, name="cu", tag="cu", bufs=4)
                nc.vector.tensor_copy(cu[:], cH[:])
                z_sb = miscp.tile([1, N], f32, name="z_sb", tag="z", bufs=1)
                nc.vector.tensor_copy(z_sb[:], cu[64:65, :])
                rz = miscp.tile([1, N], f32, name="rz", tag="rz", bufs=2)
                nc.vector.reciprocal_approx_fast(rz[:], z_sb[:])
                rzb = miscp.tile([64, N], f32, name="rzb", tag="rzb", bufs=3)
                nc.gpsimd.partition_broadcast(rzb[:], rz[0:1, :])

                def mult(cn=cn, hl=hl, cu=cu, rzb=rzb):
                    nc.vector.tensor_tensor(
                        cn[hl * 64:(hl + 1) * 64, :],
                        cu[0:64, :],
                        rzb[:],
                        MULT,
                    )
                return mult

            def emit_pack(u, fillers):
                b, p = u
                qst, kst = qks[b][p][0], qks[b][p][1]
                pts[u] = {}
                for jc in range(8):
                    js = slice(jc * 128, (jc + 1) * 128)
                    for ic in range(2):
                        isl = slice(ic * 512, (ic + 1) * 512)
                        sp = psum.tile([128, 2, 512], f32, name="sp",
                                       tag=f"sp{ic}")
                        for hl in range(2):
                            hs = slice(hl * 64, (hl + 1) * 64)
                            nc.tensor.matmul(
                                sp[:, hl, :],
                                kst[hs, js],
                                qst[hs, isl],
                                start=True, stop=True,
                                tile_position=(hl * 64, 0),
                            )
                        pt = pp.tile([128, 2, 512], bf16, name="pt", tag="pt")
                        nc.scalar.activation(pt[:], sp[:], EXP, scale=SCALE)
                        pts[u][(jc, ic)] = pt
                    for f in fillers[jc]:
                        f()

            def chainf(*fns):
                return [f for f in fns if f is not None]

            units = [(b, p) for b in range(BPC) for p in range(2)]

            # prologue
            emit_x_load(0)
            emit_qk_full(0, 0, 0, tag="sp0")
            emit_qk_full(0, 0, 1, tag="sp1")
            for pr in range(4):
                emit_v_pair(0, pr)
            emit_x_load(1)

            mult_q = []   # deferred normalize multiplies
            for ui, u in enumerate(units):
                b, p = u
                prev_u = units[ui - 1] if ui >= 1 else None
                nxt_b = b + 1 if b + 1 < BPC else None

                def rp(hl, part, pu=prev_u):
                    if pu is None:
                        return None
                    return lambda: replay_mms(pu, hl, part)

                def na(hl, pu=prev_u):
                    if pu is None:
                        return None
                    return lambda: mult_q.append(norm_a(pu, hl))

                def dm():
                    # drain one deferred multiply
                    return (lambda: mult_q.pop(0)()) if mult_q else None

                if p == 0:
                    projf = [
                        None,
                        None,
                        (lambda _b=b: emit_qk_full(_b, 1, 0)),
                        (lambda _b=b: emit_qk_full(_b, 1, 1)),
                        (lambda _n=nxt_b: emit_v_pair(_n, 0)) if nxt_b is not None else None,
                        None,
                        (lambda _n=nxt_b: emit_v_pair(_n, 1)) if nxt_b is not None else None,
                        None,
                    ]
                else:
                    prv_b = b - 1 if b >= 1 else None
                    last_u = (ui == len(units) - 1)
                    projf = [
                        None,
                        None,
                        (lambda _p=prv_b: emit_outproj_full(_p, 0)) if prv_b is not None else None,
                        (lambda _p=prv_b: emit_outproj_full(_p, 1)) if prv_b is not None else None,
                        (lambda _n=nxt_b: emit_v_pair(_n, 2)) if nxt_b is not None else None,
                        (lambda _n=nxt_b: emit_qk_full(_n, 0, 0)) if nxt_b is not None else None,
                        (lambda _n=nxt_b: emit_v_pair(_n, 3)) if nxt_b is not None else None,
                        (lambda _n=nxt_b: emit_qk_full(_n, 0, 1)) if nxt_b is not None else None,
                    ]
                    if last_u:
                        # overlap the final unit's h0 replay with its own
                        # attention stream, in the now-idle fq slot
                        projf[4] = (lambda _u=u: replay_mms(_u, 0, 0, tag="fq"))
                        projf[5] = (lambda _u=u: replay_mms(_u, 0, 1, tag="fq"))
                        projf[6] = (lambda _u=u: replay_mms(_u, 0, 2, tag="fq"))
                        projf[7] = (lambda _u=u: [replay_mms(_u, 0, 3, tag="fq"),
                                                  mult_q.append(norm_a(_u, 0, tag="fq"))])

                fillers = [
                    chainf(rp(0, 0), dm(), projf[0]),
                    chainf(rp(0, 1), dm(), projf[1]),
                    chainf(rp(0, 2), projf[2]),
                    chainf(rp(0, 3), na(0), projf[3]),
                    chainf(rp(1, 0), projf[4]),
                    chainf(rp(1, 1), projf[5]),
                    chainf(rp(1, 2), projf[6]),
                    chainf(rp(1, 3), na(1), projf[7]),
                ]
                emit_pack(u, fillers)
                if p == 1 and b + 2 < BPC:
                    emit_x_load(b + 2)

            # tail: replay the final unit, finish norms, last out projection
            last = units[-1]
            for part in range(4):
                replay_mms(last, 1, part)
            mult_q.append(norm_a(last, 1))
            for f in mult_q:
                f()
            for co in range(2):
                emit_outproj_full(BPC - 1, co)

    nc.compile()
    return nc


def _prep_weights(proj_w, proj_b, out_w, out_b):
    qk_cols = []
    for p in range(2):
        for qk in range(2):
            for hl in range(2):
                h = 2 * p + hl
                base = h * 192 + qk * 64
                qk_cols.extend(range(base, base + 64))
    wqk = np.ascontiguousarray(proj_w[qk_cols, :].T).astype(ml_dtypes.bfloat16)
    bqk = np.ascontiguousarray(proj_b[qk_cols].reshape(4, 128).T)

    wv = np.zeros((C, 260), dtype=np.float32)
    wvb1 = np.zeros((1, 260), dtype=np.float32)
    for h in range(N_HEADS):
        rows = range(h * 192 + 128, h * 192 + 192)
        wv[:, h * 65:h * 65 + 64] = proj_w[rows, :].T
        wvb1[0, h * 65:h * 65 + 64] = proj_b[rows]
        wvb1[0, h * 65 + 64] = 1.0
    wv = wv.astype(ml_dtypes.bfloat16)
    wvb2 = np.ascontiguousarray(
        np.broadcast_to(np.concatenate([wvb1, wvb1], axis=1), (128, 520))
    )

    wo = np.ascontiguousarray(out_w.T).astype(ml_dtypes.bfloat16)
    ob = np.ascontiguousarray(out_b.reshape(2, 128).T)
    return dict(wqk=wqk, bqk=bqk, wv=wv, wvb2=wvb2, wo=wo, ob=ob)


def kernel(x, proj_w, proj_b, out_w, out_b, _trace=False):
    from concourse.bass_utils import run_bass_kernel_spmd

    x = np.asarray(x, dtype=np.float32)
    proj_w = np.asarray(proj_w, dtype=np.float32)
    proj_b = np.asarray(proj_b, dtype=np.float32)
    out_w = np.asarray(out_w, dtype=np.float32)
    out_b = np.asarray(out_b, dtype=np.float32)

    if "nc" not in _CACHE:
        _CACHE["nc"] = _build()
    nc = _CACHE["nc"]

    w = _prep_weights(proj_w, proj_b, out_w, out_b)
    xs = np.ascontiguousarray(x.reshape(B, C, N))
    xsbf = xs.astype(ml_dtypes.bfloat16)
    in_maps = [
        dict(w, x=np.ascontiguousarray(xs[i * BPC:(i + 1) * BPC]),
             xbf=np.ascontiguousarray(xsbf[i * BPC:(i + 1) * BPC]))
        for i in range(NCORES)
    ]
    res = run_bass_kernel_spmd(nc, in_maps, core_ids=list(range(NCORES)), trace=_trace)
    out = np.concatenate([r["out"] for r in res.results], axis=0)
    out = out.reshape(B, C, H, W)
    if _trace:
        _CACHE["last_result"] = res
    return out


# revision 45
# speedup vs baseline: 1.0280x; 1.0274x over previous
"""AttnBlock Trainium2 Bass kernel.

Data-parallel over batch across 8 NeuronCores (4 batch elements each, full
weights on every core). Everything on-chip is feature-major ([feat, token]),
so the pipeline needs no transposes anywhere.

The kernel is paced by the Scalar engine: softmax exp is 16.8M elements per
core and ACT runs 1 elem/lane/cycle @1.2GHz => ~147us floor (128 ACT ops).
The whole design exists to keep that stream gapless:

  - scores: sp [128, 2, 512] psum, both heads x one i-half, head pair
    row-tiled (tile_position (0,0)/(64,0)) so it streams concurrently;
    two sp tiles ping-pong. NOTHING else ever touches the sp slots, so
    the scores->exp stream has no foreign WAR waits.
  - ctx: ONE [65, N] psum accumulator (row 64 = softmax Z via the ones
    column of V). Each attention unit's ctx matmuls are REPLAYED from the
    SBUF P tiles during the NEXT unit, spread 4 MMs per jc-slot into the
    PE's slack under the ACT pace. Head h0 replays in jc0-3, h1 in jc4-7.
  - projections (QK/V/out) run in a DEDICATED fq psum slot, emitted as
    one filler per jc-slot; their DVE consumers chain only to each other.
  - normalize: copies + reciprocal (pure DVE) right at each head's replay
    end; the GPSIMD broadcast then fills; the multiplies are deferred two
    jc-slots so they never wait on GPSIMD from inside the DVE FIFO.

PSUM (8 banks, exactly full): sp0, sp1 (4), cH (2), fq (2).

Matmul operands are bf16 (converted host-side; fp32 PSUM accumulation).
"""

import numpy as np
import ml_dtypes

N_HEADS = 4
D_K = 64
SCALE = D_K ** (-0.5)
B, C, H, W = 32, 256, 32, 32
N = H * W           # 1024 tokens
NCORES = 8
BPC = B // NCORES   # 4 batch elements per core

_CACHE = {}


def _build():
    import concourse.bacc as bacc
    import concourse.mybir as mybir
    from concourse.tile import TileContext

    dt = mybir.dt
    f32 = dt.float32
    bf16 = dt.bfloat16
    EXP = mybir.ActivationFunctionType.Exp
    ADD = mybir.AluOpType.add
    MULT = mybir.AluOpType.mult

    nc = bacc.Bacc()
    x = nc.dram_tensor("x", [BPC, C, N], f32, kind="ExternalInput")
    xbf = nc.dram_tensor("xbf", [BPC, C, N], bf16, kind="ExternalInput")
    wqk = nc.dram_tensor("wqk", [C, 512], bf16, kind="ExternalInput")
    bqk = nc.dram_tensor("bqk", [128, 4], f32, kind="ExternalInput")
    wv = nc.dram_tensor("wv", [C, 260], bf16, kind="ExternalInput")
    wvb2 = nc.dram_tensor("wvb2", [128, 520], f32, kind="ExternalInput")
    wo = nc.dram_tensor("wo", [C, C], bf16, kind="ExternalInput")
    ob = nc.dram_tensor("ob", [128, 2], f32, kind="ExternalInput")
    out = nc.dram_tensor("out", [BPC, C, N], f32, kind="ExternalOutput")

    with TileContext(nc) as tc:
        with (
            tc.tile_pool(name="consts", bufs=1) as consts,
            tc.tile_pool(name="xp", bufs=4) as xp,
            tc.tile_pool(name="qkp", bufs=5) as qkp,
            tc.tile_pool(name="vp", bufs=3) as vp,
            tc.tile_pool(name="pp", bufs=26) as pp,
            tc.tile_pool(name="miscp", bufs=4) as miscp,
            tc.tile_pool(name="outp", bufs=2) as outp,
            tc.tile_pool(name="psum", bufs=1, space="PSUM") as psum,
        ):
            wqk_sb = [consts.tile([128, 512], bf16, name=f"wqk{cc}") for cc in range(2)]
            wv_sb = [consts.tile([128, 260], bf16, name=f"wv{cc}") for cc in range(2)]
            wo_sb = [consts.tile([128, 256], bf16, name=f"wo{cc}") for cc in range(2)]
            bqk_sb = consts.tile([128, 4], f32, name="bqk_sb")
            wvb2_sb = consts.tile([128, 520], f32, name="wvb2_sb")
            ob_sb = consts.tile([128, 2], f32, name="ob_sb")
            xcs, xcrs, qks, vss, cns, pts = {}, {}, {}, {}, {}, {}
            _dma_rest = []

            nc.sync.dma_start(out=bqk_sb[:], in_=bqk[:])
            for cc in range(2):
                nc.sync.dma_start(out=wqk_sb[cc][:], in_=wqk[cc * 128:(cc + 1) * 128, :])

            def _dma_consts_rest():
                for cc in range(2):
                    nc.sync.dma_start(out=wv_sb[cc][:], in_=wv[cc * 128:(cc + 1) * 128, :])
                nc.sync.dma_start(out=wvb2_sb[:], in_=wvb2[:])
                for cc in range(2):
                    nc.sync.dma_start(out=wo_sb[cc][:], in_=wo[cc * 128:(cc + 1) * 128, :])
                nc.sync.dma_start(out=ob_sb[:], in_=ob[:])
            warmup = consts.tile([1, 4], f32, name="warmup")
            nc.scalar.activation(warmup[:], bqk_sb[0:1, 0:4], EXP)

            def emit_xcr_load(b):
                xcr = [xp.tile([128, N], bf16, name=f"xcr{cc}", tag=f"xcr{cc}", bufs=3)
                       for cc in range(2)]
                for cc in range(2):
                    nc.sync.dma_start(out=xcr[cc][:], in_=xbf[b, cc * 128:(cc + 1) * 128, :])
                xcrs[b] = xcr

            def emit_xc_load(b):
                xc = [xp.tile([128, N], f32, name=f"xc{cc}", tag=f"xc{cc}", bufs=3)
                      for cc in range(2)]
                for cc in range(2):
                    nc.sync.dma_start(out=xc[cc][:], in_=x[b, cc * 128:(cc + 1) * 128, :])
                xcs[b] = xc

            def emit_x_load(b):
                emit_xcr_load(b)
                emit_xc_load(b)

            def emit_qk_full(b, p, qk, tag="fq"):
                if b not in qks:
                    qks[b] = [[None, None], [None, None]]
                xcr = xcrs[b]
                qkps = psum.tile([128, N], f32, name="qkps", tag=tag)
                col0 = p * 256 + qk * 128
                for fc in range(2):
                    fs = slice(fc * 512, (fc + 1) * 512)
                    for cc in range(2):
                        nc.tensor.matmul(
                            qkps[:, fs],
                            wqk_sb[cc][:, col0:col0 + 128],
                            xcr[cc][:, fs],
                            start=(cc == 0), stop=(cc == 1),
                        )
                qks[b][p][qk] = qkp.tile([128, N], bf16, name=f"qk{p}{qk}")
                nc.vector.tensor_scalar(
                    qks[b][p][qk][:], qkps[:],
                    bqk_sb[:, 2 * p + qk:2 * p + qk + 1],
                    None, ADD,
                )

            def emit_v_pair(b, pr, tag="fq"):
                if b not in vss:
                    vss[b] = vp.tile([128, 8, 260], bf16, name="v_sb", tag="v")
                xcr = xcrs[b]
                vps = psum.tile([128, 2, 512], f32, name="vps", tag=tag)
                for k in range(2):
                    jt = 2 * pr + k
                    js = slice(jt * 128, (jt + 1) * 128)
                    for cc in range(2):
                        nc.tensor.matmul(
                            vps[:, k, 0:260],
                            xcr[cc][:, js], wv_sb[cc][:],
                            start=(cc == 0), stop=(cc == 1),
                        )
                nc.vector.scalar_tensor_tensor(
                    vss[b][:, 2 * pr:2 * pr + 2, :], vps[:, :, 0:260],
                    1.0, wvb2_sb[:], MULT, ADD,
                )

            def emit_outproj_full(b, co):
                osb = outp.tile([128, N], f32, name="osb")
                ctxn = cns[b]
                ops = psum.tile([128, N], f32, name="ops", tag="fq")
                for fc in range(2):
                    fs = slice(fc * 512, (fc + 1) * 512)
                    for kc in range(2):
                        nc.tensor.matmul(
                            ops[:, fs],
                            wo_sb[kc][:, co * 128:(co + 1) * 128],
                            ctxn[kc][:, fs],
                            start=(kc == 0), stop=(kc == 1),
                        )
                nc.vector.scalar_tensor_tensor(
                    osb[:], ops[:], ob_sb[:, co:co + 1], xcs[b][co][:],
                    ADD, ADD,
                )
                nc.sync.dma_start(
                    out=out[b, co * 128:(co + 1) * 128, :], in_=osb[:]
                )

            # ---- ctx replay machinery -------------------------------------
            # replaying unit u=(b,p): head h's 16 ctx MMs accumulate into a
            # fresh cH tile; afterwards phase-A normalize (copies+recip,
            # pure DVE) releases cH and seeds the GPSIMD broadcast; the
            # multiply closure is returned for deferred emission.
            state = {}

            def replay_mms(u, hl, part, tag="c"):
                b, p = u
                h = 2 * p + hl
                v_sb = vss[b]
                if part == 0:
                    state[tag] = psum.tile([65, N], f32, name="cH", tag=tag)
                cH = state[tag]
                for k in range(4):
                    idx = part * 4 + k
                    jc, ic = idx // 2, idx % 2
                    isl = slice(ic * 512, (ic + 1) * 512)
                    nc.tensor.matmul(
                        cH[:, isl],
                        v_sb[:, jc, h * 65:(h + 1) * 65],
                        pts[u][(jc, ic)][:, hl, :],
                        start=(jc == 0), stop=(jc == 7),
                    )

            def norm_a(u, hl, tag="c"):
                b, p = u
                if hl == 0:
                    cns[u[0]] = cns.get(u[0]) or [None, None]
                if cns[b][p] is None:
                    cns[b][p] = miscp.tile([128, N], bf16, name=f"cn{p}",
                                           tag="cn", bufs=3)
                cn = cns[b][p]
                cH = state[tag]
                cu = miscp.tile([65, N], f32, name="cu", tag="cu", bufs=4)
                nc.vector.tensor_copy(cu[:], cH[:])
                z_sb = miscp.tile([1, N], f32, name="z_sb", tag="z", bufs=1)
                nc.vector.tensor_copy(z_sb[:], cu[64:65, :])
                rz = miscp.tile([1, N], f32, name="rz", tag="rz", bufs=2)
                nc.vector.reciprocal_approx_fast(rz[:], z_sb[:])
                rzb = miscp.tile([64, N], f32, name="rzb", tag="rzb", bufs=3)
                nc.gpsimd.partition_broadcast(rzb[:], rz[0:1, :])

                def mult(cn=cn, hl=hl, cu=cu, rzb=rzb):
                    nc.vector.tensor_tensor(
                        cn[hl * 64:(hl + 1) * 64, :],
                        cu[0:64, :],
                        rzb[:],
                        MULT,
                    )
                return mult

            def emit_pack(u, fillers):
                b, p = u
                qst, kst = qks[b][p][0], qks[b][p][1]
                pts[u] = {}
                for jc in range(8):
                    js = slice(jc * 128, (jc + 1) * 128)
                    for ic in range(2):
                        isl = slice(ic * 512, (ic + 1) * 512)
                        sp = psum.tile([128, 2, 512], f32, name="sp",
                                       tag=f"sp{ic}")
                        for hl in range(2):
                            hs = slice(hl * 64, (hl + 1) * 64)
                            nc.tensor.matmul(
                                sp[:, hl, :],
                                kst[hs, js],
                                qst[hs, isl],
                                start=True, stop=True,
                                tile_position=(hl * 64, 0),
                            )
                        pt = pp.tile([128, 2, 512], bf16, name="pt", tag="pt")
                        nc.scalar.activation(pt[:], sp[:], EXP, scale=SCALE)
                        pts[u][(jc, ic)] = pt
                    for f in fillers[jc]:
                        f()

            def chainf(*fns):
                return [f for f in fns if f is not None]

            units = [(b, p) for b in range(BPC) for p in range(2)]

            # prologue: xcr(0) DMA fires before the bulky consts so the
            # first QK projection (and hence the exp stream) starts ASAP
            emit_xcr_load(0)
            _dma_consts_rest()
            emit_qk_full(0, 0, 0, tag="sp0")
            emit_qk_full(0, 0, 1, tag="sp1")
            for pr in range(4):
                emit_v_pair(0, pr)
            emit_xc_load(0)
            emit_x_load(1)

            mult_q = []   # deferred normalize multiplies
            for ui, u in enumerate(units):
                b, p = u
                prev_u = units[ui - 1] if ui >= 1 else None
                nxt_b = b + 1 if b + 1 < BPC else None

                def rp(hl, part, pu=prev_u):
                    if pu is None:
                        return None
                    return lambda: replay_mms(pu, hl, part)

                def na(hl, pu=prev_u):
                    if pu is None:
                        return None
                    return lambda: mult_q.append(norm_a(pu, hl))

                def dm():
                    # drain one deferred multiply
                    return (lambda: mult_q.pop(0)()) if mult_q else None

                if p == 0:
                    projf = [
                        None,
                        None,
                        (lambda _b=b: emit_qk_full(_b, 1, 0)),
                        (lambda _b=b: emit_qk_full(_b, 1, 1)),
                        (lambda _n=nxt_b: emit_v_pair(_n, 0)) if nxt_b is not None else None,
                        None,
                        (lambda _n=nxt_b: emit_v_pair(_n, 1)) if nxt_b is not None else None,
                        None,
                    ]
                else:
                    prv_b = b - 1 if b >= 1 else None
                    last_u = (ui == len(units) - 1)
                    projf = [
                        None,
                        None,
                        (lambda _p=prv_b: emit_outproj_full(_p, 0)) if prv_b is not None else None,
                        (lambda _p=prv_b: emit_outproj_full(_p, 1)) if prv_b is not None else None,
                        (lambda _n=nxt_b: emit_v_pair(_n, 2)) if nxt_b is not None else None,
                        (lambda _n=nxt_b: emit_qk_full(_n, 0, 0)) if nxt_b is not None else None,
                        (lambda _n=nxt_b: emit_v_pair(_n, 3)) if nxt_b is not None else None,
                        (lambda _n=nxt_b: emit_qk_full(_n, 0, 1)) if nxt_b is not None else None,
                    ]
                    if last_u:
                        # overlap the final unit's h0 replay with its own
                        # attention stream, in the now-idle fq slot
                        projf[4] = (lambda _u=u: replay_mms(_u, 0, 0, tag="fq"))
                        projf[5] = (lambda _u=u: replay_mms(_u, 0, 1, tag="fq"))
                        projf[6] = (lambda _u=u: replay_mms(_u, 0, 2, tag="fq"))
                        projf[7] = (lambda _u=u: [replay_mms(_u, 0, 3, tag="fq"),
                                                  mult_q.append(norm_a(_u, 0, tag="fq"))])

                fillers = [
                    chainf(rp(0, 0), dm(), projf[0]),
                    chainf(rp(0, 1), dm(), projf[1]),
                    chainf(rp(0, 2), projf[2]),
                    chainf(rp(0, 3), na(0), projf[3]),
                    chainf(rp(1, 0), projf[4]),
                    chainf(rp(1, 1), projf[5]),
                    chainf(rp(1, 2), projf[6]),
                    chainf(rp(1, 3), na(1), projf[7]),
                ]
                emit_pack(u, fillers)
                if p == 1 and b + 2 < BPC:
                    emit_x_load(b + 2)

            # tail: replay the final unit, finish norms, last out projection
            last = units[-1]
            for part in range(4):
                replay_mms(last, 1, part)
            mult_q.append(norm_a(last, 1))
            for f in mult_q:
                f()
            for co in range(2):
                emit_outproj_full(BPC - 1, co)

    nc.compile()
    return nc


def _prep_weights(proj_w, proj_b, out_w, out_b):
    qk_cols = []
    for p in range(2):
        for qk in range(2):
            for hl in range(2):
                h = 2 * p + hl
                base = h * 192 + qk * 64
                qk_cols.extend(range(base, base + 64))
    wqk = np.ascontiguousarray(proj_w[qk_cols, :].T).astype(ml_dtypes.bfloat16)
    bqk = np.ascontiguousarray(proj_b[qk_cols].reshape(4, 128).T)

    wv = np.zeros((C, 260), dtype=np.float32)
    wvb1 = np.zeros((1, 260), dtype=np.float32)
    for h in range(N_HEADS):
        rows = range(h * 192 + 128, h * 192 + 192)
        wv[:, h * 65:h * 65 + 64] = proj_w[rows, :].T
        wvb1[0, h * 65:h * 65 + 64] = proj_b[rows]
        wvb1[0, h * 65 + 64] = 1.0
    wv = wv.astype(ml_dtypes.bfloat16)
    wvb2 = np.ascontiguousarray(
        np.broadcast_to(np.concatenate([wvb1, wvb1], axis=1), (128, 520))
    )

    wo = np.ascontiguousarray(out_w.T).astype(ml_dtypes.bfloat16)
    ob = np.ascontiguousarray(out_b.reshape(2, 128).T)
    return dict(wqk=wqk, bqk=bqk, wv=wv, wvb2=wvb2, wo=wo, ob=ob)


def kernel(x, proj_w, proj_b, out_w, out_b, _trace=False):
    from concourse.bass_utils import run_bass_kernel_spmd

    x = np.asarray(x, dtype=np.float32)
    proj_w = np.asarray(proj_w, dtype=np.float32)
    proj_b = np.asarray(proj_b, dtype=np.float32)
    out_w = np.asarray(out_w, dtype=np.float32)
    out_b = np.asarray(out_b, dtype=np.float32)

    if "nc" not in _CACHE:
        _CACHE["nc"] = _build()
    nc = _CACHE["nc"]

    w = _prep_weights(proj_w, proj_b, out_w, out_b)
    xs = np.ascontiguousarray(x.reshape(B, C, N))
    xsbf = xs.astype(ml_dtypes.bfloat16)
    in_maps = [
        dict(w, x=np.ascontiguousarray(xs[i * BPC:(i + 1) * BPC]),
             xbf=np.ascontiguousarray(xsbf[i * BPC:(i + 1) * BPC]))
        for i in range(NCORES)
    ]
    res = run_bass_kernel_spmd(nc, in_maps, core_ids=list(range(NCORES)), trace=_trace)
    out = np.concatenate([r["out"] for r in res.results], axis=0)
    out = out.reshape(B, C, H, W)
    if _trace:
        _CACHE["last_result"] = res
    return out


# revision 46
# speedup vs baseline: 1.0300x; 1.0019x over previous
"""AttnBlock Trainium2 Bass kernel.

Data-parallel over batch across 8 NeuronCores (4 batch elements each, full
weights on every core). Everything on-chip is feature-major ([feat, token]),
so the pipeline needs no transposes anywhere.

The kernel is paced by the Scalar engine: softmax exp is 16.8M elements per
core and ACT runs 1 elem/lane/cycle @1.2GHz => ~147us floor (128 ACT ops).
The whole design exists to keep that stream gapless:

  - scores: sp [128, 2, 512] psum, both heads x one i-half, head pair
    row-tiled (tile_position (0,0)/(64,0)) so it streams concurrently;
    two sp tiles ping-pong. NOTHING else ever touches the sp slots, so
    the scores->exp stream has no foreign WAR waits.
  - ctx: ONE [65, N] psum accumulator (row 64 = softmax Z via the ones
    column of V). Each attention unit's ctx matmuls are REPLAYED from the
    SBUF P tiles during the NEXT unit, spread 4 MMs per jc-slot into the
    PE's slack under the ACT pace. Head h0 replays in jc0-3, h1 in jc4-7.
  - projections (QK/V/out) run in a DEDICATED fq psum slot, emitted as
    one filler per jc-slot; their DVE consumers chain only to each other.
  - normalize: copies + reciprocal (pure DVE) right at each head's replay
    end; the GPSIMD broadcast then fills; the multiplies are deferred two
    jc-slots so they never wait on GPSIMD from inside the DVE FIFO.

PSUM (8 banks, exactly full): sp0, sp1 (4), cH (2), fq (2).

Matmul operands are bf16 (converted host-side; fp32 PSUM accumulation).
"""

import numpy as np
import ml_dtypes

N_HEADS = 4
D_K = 64
SCALE = D_K ** (-0.5)
B, C, H, W = 32, 256, 32, 32
N = H * W           # 1024 tokens
NCORES = 8
BPC = B // NCORES   # 4 batch elements per core

_CACHE = {}


def _build():
    import concourse.bacc as bacc
    import concourse.mybir as mybir
    from concourse.tile import TileContext

    dt = mybir.dt
    f32 = dt.float32
    bf16 = dt.bfloat16
    EXP = mybir.ActivationFunctionType.Exp
    ADD = mybir.AluOpType.add
    MULT = mybir.AluOpType.mult

    nc = bacc.Bacc()
    x = nc.dram_tensor("x", [BPC, C, N], f32, kind="ExternalInput")
    xbf = nc.dram_tensor("xbf", [BPC, C, N], bf16, kind="ExternalInput")
    wqk = nc.dram_tensor("wqk", [C, 512], bf16, kind="ExternalInput")
    bqk = nc.dram_tensor("bqk", [128, 4], f32, kind="ExternalInput")
    wv = nc.dram_tensor("wv", [C, 260], bf16, kind="ExternalInput")
    wvb2 = nc.dram_tensor("wvb2", [128, 520], f32, kind="ExternalInput")
    wo = nc.dram_tensor("wo", [C, C], bf16, kind="ExternalInput")
    ob = nc.dram_tensor("ob", [128, 2], f32, kind="ExternalInput")
    out = nc.dram_tensor("out", [BPC, C, N], f32, kind="ExternalOutput")

    with TileContext(nc) as tc:
        with (
            tc.tile_pool(name="consts", bufs=1) as consts,
            tc.tile_pool(name="xp", bufs=4) as xp,
            tc.tile_pool(name="qkp", bufs=5) as qkp,
            tc.tile_pool(name="vp", bufs=3) as vp,
            tc.tile_pool(name="pp", bufs=26) as pp,
            tc.tile_pool(name="miscp", bufs=4) as miscp,
            tc.tile_pool(name="outp", bufs=2) as outp,
            tc.tile_pool(name="psum", bufs=1, space="PSUM") as psum,
        ):
            wqk_sb = [consts.tile([128, 512], bf16, name=f"wqk{cc}") for cc in range(2)]
            wv_sb = [consts.tile([128, 260], bf16, name=f"wv{cc}") for cc in range(2)]
            wo_sb = [consts.tile([128, 256], bf16, name=f"wo{cc}") for cc in range(2)]
            bqk_sb = consts.tile([128, 4], f32, name="bqk_sb")
            wvb2_sb = consts.tile([128, 520], f32, name="wvb2_sb")
            ob_sb = consts.tile([128, 2], f32, name="ob_sb")
            xcs, xcrs, qks, vss, cns, pts = {}, {}, {}, {}, {}, {}
            _dma_rest = []

            nc.sync.dma_start(out=bqk_sb[:], in_=bqk[:])
            for cc in range(2):
                nc.sync.dma_start(out=wqk_sb[cc][:], in_=wqk[cc * 128:(cc + 1) * 128, :])

            def _dma_consts_rest():
                for cc in range(2):
                    nc.sync.dma_start(out=wv_sb[cc][:], in_=wv[cc * 128:(cc + 1) * 128, :])
                nc.sync.dma_start(out=wvb2_sb[:], in_=wvb2[:])
                for cc in range(2):
                    nc.sync.dma_start(out=wo_sb[cc][:], in_=wo[cc * 128:(cc + 1) * 128, :])
                nc.sync.dma_start(out=ob_sb[:], in_=ob[:])
            warmup = consts.tile([1, 4], f32, name="warmup")
            nc.scalar.activation(warmup[:], bqk_sb[0:1, 0:4], EXP)

            def emit_xcr_load(b):
                xcr = [xp.tile([128, N], bf16, name=f"xcr{cc}", tag=f"xcr{cc}", bufs=3)
                       for cc in range(2)]
                for cc in range(2):
                    nc.sync.dma_start(out=xcr[cc][:], in_=xbf[b, cc * 128:(cc + 1) * 128, :])
                xcrs[b] = xcr

            def emit_xc_load(b):
                xc = [xp.tile([128, N], f32, name=f"xc{cc}", tag=f"xc{cc}", bufs=3)
                      for cc in range(2)]
                for cc in range(2):
                    nc.sync.dma_start(out=xc[cc][:], in_=x[b, cc * 128:(cc + 1) * 128, :])
                xcs[b] = xc

            def emit_x_load(b):
                emit_xcr_load(b)
                emit_xc_load(b)

            def emit_qk_full(b, p, qk, tag="fq"):
                if b not in qks:
                    qks[b] = [[None, None], [None, None]]
                xcr = xcrs[b]
                qkps = psum.tile([128, N], f32, name="qkps", tag=tag)
                col0 = p * 256 + qk * 128
                for fc in range(2):
                    fs = slice(fc * 512, (fc + 1) * 512)
                    for cc in range(2):
                        nc.tensor.matmul(
                            qkps[:, fs],
                            wqk_sb[cc][:, col0:col0 + 128],
                            xcr[cc][:, fs],
                            start=(cc == 0), stop=(cc == 1),
                        )
                qks[b][p][qk] = qkp.tile([128, N], bf16, name=f"qk{p}{qk}")
                nc.vector.tensor_scalar(
                    qks[b][p][qk][:], qkps[:],
                    bqk_sb[:, 2 * p + qk:2 * p + qk + 1],
                    None, ADD,
                )

            def emit_v_pair(b, pr, tag="fq"):
                if b not in vss:
                    vss[b] = vp.tile([128, 8, 260], bf16, name="v_sb", tag="v")
                xcr = xcrs[b]
                vps = psum.tile([128, 2, 512], f32, name="vps", tag=tag)
                for k in range(2):
                    jt = 2 * pr + k
                    js = slice(jt * 128, (jt + 1) * 128)
                    for cc in range(2):
                        nc.tensor.matmul(
                            vps[:, k, 0:260],
                            xcr[cc][:, js], wv_sb[cc][:],
                            start=(cc == 0), stop=(cc == 1),
                        )
                nc.vector.scalar_tensor_tensor(
                    vss[b][:, 2 * pr:2 * pr + 2, :], vps[:, :, 0:260],
                    1.0, wvb2_sb[:], MULT, ADD,
                )

            def emit_outproj_full(b, co, tag="fq"):
                osb = outp.tile([128, N], f32, name="osb")
                ctxn = cns[b]
                ops = psum.tile([128, N], f32, name="ops", tag=tag)
                for fc in range(2):
                    fs = slice(fc * 512, (fc + 1) * 512)
                    for kc in range(2):
                        nc.tensor.matmul(
                            ops[:, fs],
                            wo_sb[kc][:, co * 128:(co + 1) * 128],
                            ctxn[kc][:, fs],
                            start=(kc == 0), stop=(kc == 1),
                        )
                nc.vector.scalar_tensor_tensor(
                    osb[:], ops[:], ob_sb[:, co:co + 1], xcs[b][co][:],
                    ADD, ADD,
                )
                nc.sync.dma_start(
                    out=out[b, co * 128:(co + 1) * 128, :], in_=osb[:]
                )

            # ---- ctx replay machinery -------------------------------------
            # replaying unit u=(b,p): head h's 16 ctx MMs accumulate into a
            # fresh cH tile; afterwards phase-A normalize (copies+recip,
            # pure DVE) releases cH and seeds the GPSIMD broadcast; the
            # multiply closure is returned for deferred emission.
            state = {}

            def replay_mms(u, hl, part, tag="c"):
                b, p = u
                h = 2 * p + hl
                v_sb = vss[b]
                if part == 0:
                    state[tag] = psum.tile([65, N], f32, name="cH", tag=tag)
                cH = state[tag]
                for k in range(4):
                    idx = part * 4 + k
                    jc, ic = idx // 2, idx % 2
                    isl = slice(ic * 512, (ic + 1) * 512)
                    nc.tensor.matmul(
                        cH[:, isl],
                        v_sb[:, jc, h * 65:(h + 1) * 65],
                        pts[u][(jc, ic)][:, hl, :],
                        start=(jc == 0), stop=(jc == 7),
                    )

            def norm_a(u, hl, tag="c"):
                b, p = u
                if hl == 0:
                    cns[u[0]] = cns.get(u[0]) or [None, None]
                if cns[b][p] is None:
                    cns[b][p] = miscp.tile([128, N], bf16, name=f"cn{p}",
                                           tag="cn", bufs=3)
                cn = cns[b][p]
                cH = state[tag]
                cu = miscp.tile([65, N], f32, name="cu", tag="cu", bufs=4)
                nc.vector.tensor_copy(cu[:], cH[:])
                z_sb = miscp.tile([1, N], f32, name="z_sb", tag="z", bufs=1)
                nc.vector.tensor_copy(z_sb[:], cu[64:65, :])
                rz = miscp.tile([1, N], f32, name="rz", tag="rz", bufs=2)
                nc.vector.reciprocal_approx_fast(rz[:], z_sb[:])
                rzb = miscp.tile([64, N], f32, name="rzb", tag="rzb", bufs=3)
                nc.gpsimd.partition_broadcast(rzb[:], rz[0:1, :])

                def mult(cn=cn, hl=hl, cu=cu, rzb=rzb):
                    nc.vector.tensor_tensor(
                        cn[hl * 64:(hl + 1) * 64, :],
                        cu[0:64, :],
                        rzb[:],
                        MULT,
                    )
                return mult

            def emit_pack(u, fillers):
                b, p = u
                qst, kst = qks[b][p][0], qks[b][p][1]
                pts[u] = {}
                for jc in range(8):
                    js = slice(jc * 128, (jc + 1) * 128)
                    for ic in range(2):
                        isl = slice(ic * 512, (ic + 1) * 512)
                        sp = psum.tile([128, 2, 512], f32, name="sp",
                                       tag=f"sp{ic}")
                        for hl in range(2):
                            hs = slice(hl * 64, (hl + 1) * 64)
                            nc.tensor.matmul(
                                sp[:, hl, :],
                                kst[hs, js],
                                qst[hs, isl],
                                start=True, stop=True,
                                tile_position=(hl * 64, 0),
                            )
                        pt = pp.tile([128, 2, 512], bf16, name="pt", tag="pt")
                        nc.scalar.activation(pt[:], sp[:], EXP, scale=SCALE)
                        pts[u][(jc, ic)] = pt
                    for f in fillers[jc]:
                        f()

            def chainf(*fns):
                return [f for f in fns if f is not None]

            units = [(b, p) for b in range(BPC) for p in range(2)]

            # prologue: xcr(0) DMA fires before the bulky consts so the
            # first QK projection (and hence the exp stream) starts ASAP
            emit_xcr_load(0)
            _dma_consts_rest()
            emit_qk_full(0, 0, 0, tag="sp0")
            emit_qk_full(0, 0, 1, tag="sp1")
            for pr in range(4):
                emit_v_pair(0, pr)
            emit_xc_load(0)
            emit_x_load(1)

            mult_q = []   # deferred normalize multiplies
            for ui, u in enumerate(units):
                b, p = u
                prev_u = units[ui - 1] if ui >= 1 else None
                nxt_b = b + 1 if b + 1 < BPC else None

                def rp(hl, part, pu=prev_u):
                    if pu is None:
                        return None
                    return lambda: replay_mms(pu, hl, part)

                def na(hl, pu=prev_u):
                    if pu is None:
                        return None
                    return lambda: mult_q.append(norm_a(pu, hl))

                def dm():
                    # drain one deferred multiply
                    return (lambda: mult_q.pop(0)()) if mult_q else None

                if p == 0:
                    projf = [
                        None,
                        None,
                        (lambda _b=b: emit_qk_full(_b, 1, 0)),
                        (lambda _b=b: emit_qk_full(_b, 1, 1)),
                        (lambda _n=nxt_b: emit_v_pair(_n, 0)) if nxt_b is not None else None,
                        None,
                        (lambda _n=nxt_b: emit_v_pair(_n, 1)) if nxt_b is not None else None,
                        None,
                    ]
                else:
                    prv_b = b - 1 if b >= 1 else None
                    last_u = (ui == len(units) - 1)
                    projf = [
                        None,
                        None,
                        (lambda _p=prv_b: emit_outproj_full(_p, 0)) if prv_b is not None else None,
                        (lambda _p=prv_b: emit_outproj_full(_p, 1)) if prv_b is not None else None,
                        (lambda _n=nxt_b: emit_v_pair(_n, 2)) if nxt_b is not None else None,
                        (lambda _n=nxt_b: emit_qk_full(_n, 0, 0)) if nxt_b is not None else None,
                        (lambda _n=nxt_b: emit_v_pair(_n, 3)) if nxt_b is not None else None,
                        (lambda _n=nxt_b: emit_qk_full(_n, 0, 1)) if nxt_b is not None else None,
                    ]
                    if last_u:
                        # overlap the final unit's h0 replay with its own
                        # attention stream, in the now-idle fq slot
                        projf[4] = (lambda _u=u: replay_mms(_u, 0, 0, tag="fq"))
                        projf[5] = (lambda _u=u: replay_mms(_u, 0, 1, tag="fq"))
                        projf[6] = (lambda _u=u: replay_mms(_u, 0, 2, tag="fq"))
                        projf[7] = (lambda _u=u: [replay_mms(_u, 0, 3, tag="fq"),
                                                  mult_q.append(norm_a(_u, 0, tag="fq"))])

                fillers = [
                    chainf(rp(0, 0), dm(), projf[0]),
                    chainf(rp(0, 1), dm(), projf[1]),
                    chainf(rp(0, 2), projf[2]),
                    chainf(rp(0, 3), na(0), projf[3]),
                    chainf(rp(1, 0), projf[4]),
                    chainf(rp(1, 1), projf[5]),
                    chainf(rp(1, 2), projf[6]),
                    chainf(rp(1, 3), na(1), projf[7]),
                ]
                emit_pack(u, fillers)
                if p == 1 and b + 2 < BPC:
                    emit_x_load(b + 2)

            # tail: replay the final unit, finish norms, last out projection
            last = units[-1]
            for part in range(4):
                replay_mms(last, 1, part)
            mult_q.append(norm_a(last, 1))
            for f in mult_q:
                f()
            emit_outproj_full(BPC - 1, 0)
            emit_outproj_full(BPC - 1, 1, tag="sp0")

    nc.compile()
    return nc


def _prep_weights(proj_w, proj_b, out_w, out_b):
    qk_cols = []
    for p in range(2):
        for qk in range(2):
            for hl in range(2):
                h = 2 * p + hl
                base = h * 192 + qk * 64
                qk_cols.extend(range(base, base + 64))
    wqk = np.ascontiguousarray(proj_w[qk_cols, :].T).astype(ml_dtypes.bfloat16)
    bqk = np.ascontiguousarray(proj_b[qk_cols].reshape(4, 128).T)

    wv = np.zeros((C, 260), dtype=np.float32)
    wvb1 = np.zeros((1, 260), dtype=np.float32)
    for h in range(N_HEADS):
        rows = range(h * 192 + 128, h * 192 + 192)
        wv[:, h * 65:h * 65 + 64] = proj_w[rows, :].T
        wvb1[0, h * 65:h * 65 + 64] = proj_b[rows]
        wvb1[0, h * 65 + 64] = 1.0
    wv = wv.astype(ml_dtypes.bfloat16)
    wvb2 = np.ascontiguousarray(
        np.broadcast_to(np.concatenate([wvb1, wvb1], axis=1), (128, 520))
    )

    wo = np.ascontiguousarray(out_w.T).astype(ml_dtypes.bfloat16)
    ob = np.ascontiguousarray(out_b.reshape(2, 128).T)
    return dict(wqk=wqk, bqk=bqk, wv=wv, wvb2=wvb2, wo=wo, ob=ob)


def kernel(x, proj_w, proj_b, out_w, out_b, _trace=False):
    from concourse.bass_utils import run_bass_kernel_spmd

    x = np.asarray(x, dtype=np.float32)
    proj_w = np.asarray(proj_w, dtype=np.float32)
    proj_b = np.asarray(proj_b, dtype=np.float32)
    out_w = np.asarray(out_w, dtype=np.float32)
    out_b = np.asarray(out_b, dtype=np.float32)

    if "nc" not in _CACHE:
        _CACHE["nc"] = _build()
    nc = _CACHE["nc"]

    w = _prep_weights(proj_w, proj_b, out_w, out_b)
    xs = np.ascontiguousarray(x.reshape(B, C, N))
    xsbf = xs.astype(ml_dtypes.bfloat16)
    in_maps = [
        dict(w, x=np.ascontiguousarray(xs[i * BPC:(i + 1) * BPC]),
             xbf=np.ascontiguousarray(xsbf[i * BPC:(i + 1) * BPC]))
        for i in range(NCORES)
    ]
    res = run_bass_kernel_spmd(nc, in_maps, core_ids=list(range(NCORES)), trace=_trace)
    out = np.concatenate([r["out"] for r in res.results], axis=0)
    out = out.reshape(B, C, H, W)
    if _trace:
        _CACHE["last_result"] = res
    return out
